# revision 36
# baseline (speedup 1.0000x reference)
"""Causal self-attention (B=4, T=2048, D=1024, H=16, hd=64) on 8 trn2 NeuronCores.

Sharding: data parallel over batch (4) x tensor parallel over heads (2 groups
of 8). Core c handles batch c//2 and heads (c%2)*8 .. (c%2)*8+8.
Wq/Wk/Wv are column-parallel by head group, Wo row-parallel; the pair of
cores sharing a batch produce partial outputs that are summed on the host.

On-device layout (per core) is fully "transposed": projections produce
Q^T, K^T [512, 2048] and V [2048, 512], scores are computed as
S^T = K Q^T (j=key on partitions, i=query on free dim), softmax uses
exp without max subtraction (scores are O(6) here), the denominator
comes for free from a ones-column appended to V, and attention output
O^T [hd, T] feeds the row-parallel out-projection directly as lhsT.

The V bias never touches the device: softmax rows sum to 1, so
attn @ (V + bv) = attn @ V + bv, and bv @ Wo folds into the host-side
output bias.

Head pairs share one [128, 1024] exp; their S^T matmuls row-pack onto
the PE concurrently (partition offsets 0/64). Score matmuls and exps are
trimmed to causally-valid columns. The per-chunk emission is
software-pipelined (S of tile jt+1 ahead of AV of tile jt in the PE
stream) so the PE never waits on the scalar engine's exp; AV-B lags
AV-A by two j-tiles so the psum-slot WAR against the previous head
pair's normalize never stalls the PE.
"""

import contextlib
import ctypes
import sys
import types

import numpy as np

B, T, D = 4, 2048, 1024
H_TOT, HD = 16, 64
SCALE = HD ** -0.5
P = 128
NH = 8            # heads per core
QD = NH * HD      # 512, projected dim per core
KT = D // P       # 8 contraction tiles for projections
MT = QD // P      # 4 qdim tiles
TT = T // P       # 16 token tiles
ACH = 512         # phase-A1 token chunk (Q/K); PSUM bank caps matmul N at 512
NACH = T // ACH   # 4
ICH = 512         # attention query chunk
NIC = T // ICH    # 4

_PROGRAM = None  # compiled program cache — build once per process


def _install_ntff_hook():
    """antenv.axon_hooks is missing in this image; recreate it so
    run_bass_kernel_spmd(trace=True) can profile. Harmless if unused."""
    if "antenv.axon_hooks" in sys.modules:
        return
    try:
        import antenv
    except ImportError:
        return
    mod = types.ModuleType("antenv.axon_hooks")
    _hook = [None]
    mod.set_axon_ntff_profile_hook = lambda h: _hook.__setitem__(0, h)
    mod.get_axon_ntff_profile_hook = lambda: _hook[0]
    antenv.axon_hooks = mod
    sys.modules["antenv.axon_hooks"] = mod
    try:
        lib = ctypes.CDLL("/opt/axon/libaxon_pjrt.so")
        if not hasattr(lib, "axon_start_nrt_profile"):
            return
        lib.axon_start_nrt_profile.argtypes = [
            ctypes.POINTER(ctypes.c_int64), ctypes.c_size_t]
        lib.axon_start_nrt_profile.restype = ctypes.c_int64
        lib.axon_stop_nrt_profile.argtypes = [ctypes.c_char_p]
        lib.axon_stop_nrt_profile.restype = ctypes.c_int64

        @contextlib.contextmanager
        def _hookfn(output_dir, device_ids):
            import jax
            jax.devices()
            if device_ids:
                ids = (ctypes.c_int64 * len(device_ids))(*device_ids)
                rc = lib.axon_start_nrt_profile(ids, len(device_ids))
            else:
                rc = lib.axon_start_nrt_profile(None, 0)
            if rc != 0:
                raise RuntimeError(f"axon_start_nrt_profile rc={rc}")
            try:
                yield
            finally:
                n = lib.axon_stop_nrt_profile(str(output_dir).encode())
                print(f"profile: {n} file(s) written to {output_dir}")

        mod.set_axon_ntff_profile_hook(_hookfn)
    except OSError:
        pass


def _build_program():
    from contextlib import ExitStack

    import concourse.tile as tile
    from concourse import bacc, mybir

    F32 = mybir.dt.float32
    BF16 = mybir.dt.bfloat16
    AF = mybir.ActivationFunctionType
    ALU = mybir.AluOpType

    nc = bacc.Bacc("TRN2", target_bir_lowering=False, debug=False,
                   num_devices=8)

    # all tensor inputs arrive pre-arranged in SBUF layout [128, k, n]
    # (host does the transpose) so every DMA is long contiguous runs
    xT_d = nc.dram_tensor("xT", [P, KT * T], BF16, kind="ExternalInput").ap()
    wq_d = nc.dram_tensor("wq", [P, KT * QD], BF16, kind="ExternalInput").ap()
    wk_d = nc.dram_tensor("wk", [P, KT * QD], BF16, kind="ExternalInput").ap()
    wv_d = nc.dram_tensor("wv", [P, KT * QD], BF16, kind="ExternalInput").ap()
    wo_d = nc.dram_tensor("wo", [P, MT * D], BF16, kind="ExternalInput").ap()
    bq_d = nc.dram_tensor("bq", [P, MT], F32, kind="ExternalInput").ap()
    msk_d = nc.dram_tensor("msk", [P, P], BF16, kind="ExternalInput").ap()
    idn_d = nc.dram_tensor("idn", [P, P], BF16, kind="ExternalInput").ap()
    neg_d = nc.dram_tensor("neg", [P, P], BF16, kind="ExternalInput").ap()
    out_d = nc.dram_tensor("out", [T, D], F32, kind="ExternalOutput").ap()

    xT_k = xT_d.rearrange("p (k t) -> p k t", k=KT)      # [128, 8, 2048]
    wq_k = wq_d.rearrange("p (k m) -> p k m", k=KT)      # [128, 8, 512]
    wk_k = wk_d.rearrange("p (k m) -> p k m", k=KT)
    wv_k = wv_d.rearrange("p (k m) -> p k m", k=KT)
    wo_k = wo_d.rearrange("p (k e) -> p k e", k=MT)      # [128, 4, 1024]

    with tile.TileContext(nc) as tc, ExitStack() as ctx:
        persist = ctx.enter_context(tc.tile_pool(name="persist", bufs=1))

        qt = [persist.tile([P, T], BF16, name=f"qt{i}") for i in range(MT)]
        kt_ = [persist.tile([P, T], BF16, name=f"kt{i}") for i in range(MT)]
        v3 = [persist.tile([P, NH, HD + 1], BF16, name=f"v3_{i}")
              for i in range(TT)]
        at = [persist.tile([P, T], BF16, name=f"at{i}") for i in range(MT)]
        xt_all = persist.tile([P, KT, T], BF16, name="xt")

        wq_sb = persist.tile([P, KT, QD], BF16, name="wq")
        wk_sb = persist.tile([P, KT, QD], BF16, name="wk")
        bq_sb = persist.tile([P, MT], F32, name="bq")
        tri_sb = persist.tile([P, P], BF16, name="tri")
        idn_sb = persist.tile([P, P], BF16, name="idn")
        neg_sb = persist.tile([P, P], BF16, name="neg")
        wv_sb = persist.tile([P, KT, QD], BF16, name="wv")
        wo_sb = persist.tile([P, MT, D], BF16, name="wo")

        # startup DMAs are issued from FOUR engine DGE queues in parallel
        # (sync/scalar/vector/gpsimd are all idle during the preamble), so
        # the first matmul's inputs and the k=1..7 tiles phase A1 streams
        # through all arrive several us earlier than a single-queue issue
        # order could deliver them. Per queue, DMAs are ordered by the time
        # phase A1 consumes them.
        # weight DMAs issue from the scalar engine's hardware DGE queue,
        # in parallel with the sync queue streaming x^T: phase A1's first
        # groups stop starving on single-queue issue order. (gpsimd would
        # be a THIRD queue but is software DGE -- ~100x slower transfers.)
        nc.scalar.dma_start(wq_sb[:, 0, 0:P], wq_k[:, 0, 0:P])
        nc.sync.dma_start(xt_all[:, 0, 0:ACH], xT_k[:, 0, 0:ACH])
        nc.scalar.dma_start(wq_sb[:, 0, P:QD], wq_k[:, 0, P:QD])
        nc.sync.dma_start(xt_all[:, 0, ACH:T // 2], xT_k[:, 0, ACH:T // 2])
        for k in range(1, KT):
            nc.scalar.dma_start(wq_sb[:, k, :], wq_k[:, k, :])
            nc.sync.dma_start(xt_all[:, k, 0:T // 2], xT_k[:, k, 0:T // 2])
        for k in range(KT):
            nc.scalar.dma_start(wk_sb[:, k, :], wk_k[:, k, :])
        nc.scalar.dma_start(bq_sb[:], bq_d)
        nc.sync.dma_start(idn_sb[:], idn_d)
        nc.sync.dma_start(neg_sb[:], neg_d)
        for k in range(KT):
            nc.sync.dma_start(xt_all[:, k, T // 2:T], xT_k[:, k, T // 2:T])
        nc.sync.dma_start(tri_sb[:], msk_d)
        nc.sync.dma_start(wv_sb[:], wv_k)
        nc.sync.dma_start(wo_sb[:], wo_k)
        for tt in range(TT):
            nc.vector.memset(v3[tt][:, :, HD:HD + 1], 1.0)

        # ---- phase A1: Q^T, K^T projections --------------------------------
        with tc.tile_pool(name="pjps1", bufs=1, space="PSUM") as pjp:
            # chunk-inner so each weight tile is loaded into the PE once
            # and reused for two 512-column chunks; two half-passes so the
            # first matmuls only wait on the first half of the xT DMA
            # bk is dropped entirely: for a fixed query, the q.bk and bq.bk
            # score terms are constant across keys, so they cancel in
            # softmax; only the Q bias survives. K^T's psum->SBUF moves go
            # on the (otherwise idle in A1) scalar engine so the DVE only
            # carries Q's bias-adds and the two move queues drain in
            # parallel at the A1->attention transition.
            for half in range(1):
                for mt in range(MT - 1):
                    for w_sb, dst, b_sb in ((wq_sb, qt, bq_sb),
                                            (wk_sb, kt_, None)):
                        ps = [pjp.tile([P, ACH], F32, name="pj", bufs=8)
                              for _ in range(2)]
                        for k in range(KT):
                            for i, nch in enumerate((2 * half, 2 * half + 1)):
                                nc.tensor.matmul(
                                    ps[i][:],
                                    w_sb[:, k, mt * P:(mt + 1) * P],
                                    xt_all[:, k, nch * ACH:(nch + 1) * ACH],
                                    start=(k == 0), stop=(k == KT - 1))
                        for i, nch in enumerate((2 * half, 2 * half + 1)):
                            csl = slice(nch * ACH, (nch + 1) * ACH)
                            if b_sb is None:
                                nc.scalar.copy(dst[mt][:, csl], ps[i][:])
                            else:
                                nc.vector.tensor_scalar_add(dst[mt][:, csl],
                                                            ps[i][:],
                                                            b_sb[:, mt:mt + 1])
            # first V tiles inside the A1 psum epoch: their psum slot's
            # previous user retired 8 allocations ago, so no pool-close
            # barrier stalls the PE at the A1->attention transition
            for tt in range(4):
                psv = pjp.tile([P, ACH], F32, name="pj", bufs=8)
                for k in range(KT):
                    nc.tensor.matmul(
                        psv[:], xt_all[:, k, tt * P:(tt + 1) * P],
                        wv_sb[:, k, :], start=(k == 0), stop=(k == KT - 1))
                nc.scalar.copy(
                    v3[tt][:, :, 0:HD],
                    psv[:].rearrange("p (h d) -> p h d", d=HD))

        # ---- phases A2/B/C interleaved per query chunk ---------------------
        with tc.tile_pool(name="attnsb", bufs=1) as ap_, \
             tc.tile_pool(name="obp", bufs=8) as obp, \
             tc.tile_pool(name="attnps", bufs=1, space="PSUM") as sp:

            def emit_v_tile(tt):
                psv = sp.tile([P, QD], F32, name="misc", bufs=1)
                for k in range(KT):
                    nc.tensor.matmul(
                        psv[:], xt_all[:, k, tt * P:(tt + 1) * P],
                        wv_sb[:, k, :], start=(k == 0), stop=(k == KT - 1))
                nc.vector.tensor_copy(
                    v3[tt][:, :, 0:HD],
                    psv[:].rearrange("p (h d) -> p h d", d=HD))

            def a1h1_group(mt, w, nch):
                # second-half Q^T/K^T projection (token cols 1024:2048),
                # hoisted out of the serial phase-A1 and run as attention
                # fillers: the attention phase is scalar(exp)-bound, so
                # this PE work hides entirely under the exp stream. The
                # psum->SBUF moves go on the DVE (the scalar engine is the
                # critical path here, unlike in phase A1).
                ps = sp.tile([P, ACH], F32, name="misc", bufs=1)
                w_sb = wq_sb if w == 'q' else wk_sb
                for k in range(KT):
                    nc.tensor.matmul(
                        ps[:], w_sb[:, k, mt * P:(mt + 1) * P],
                        xt_all[:, k, nch * ACH:(nch + 1) * ACH],
                        start=(k == 0), stop=(k == KT - 1))
                csl = slice(nch * ACH, (nch + 1) * ACH)
                if w == 'q':
                    nc.vector.tensor_scalar_add(qt[mt][:, csl], ps[:],
                                                bq_sb[:, mt:mt + 1])
                else:
                    nc.vector.tensor_copy(kt_[mt][:, csl], ps[:])

            def emit_attn_chunk(ic):
                """Attention for query chunk ic as ONE flat software
                pipeline over all (head-pair, j-tile) pairs: S of tile
                g+1 is emitted before AV of tile g so exp latency is
                hidden, and the pipeline flows STRAIGHT ACROSS head-pair
                boundaries -- the next pair's scores enter the PE stream
                while the previous pair's AV-B tail drains, so the scalar
                engine (the critical path) never starves at a boundary.
                AV-B lags AV-A by four tiles so the opsum-slot WAR
                against the previous head pair's normalize clears before
                the PE reaches the first AV-B. Each normalize's
                reciprocal+broadcast are emitted as soon as its ops
                accumulation stops; its multiplies are deferred exactly
                one tile so they queue on the DVE behind the next tile's
                masks (any earlier and they would stall the mask feed,
                any later and the opsum slot's next user would outrun
                them)."""
                njt = 4 * ic + 4
                N = MT * njt
                pending_mults = []
                ops = {}
                s2s, e2s = {}, {}

                def emit_s(g):
                    hp, jt = divmod(g, njt)
                    if jt == 0:
                        # allocated full-bank so the final out-projection
                        # waves can reuse freed opsum slots with an
                        # identical tag shape
                        ops[hp] = (
                            sp.tile([P, ICH], F32, name="opsum",
                                    bufs=3)[0:HD + 1, :],
                            sp.tile([P, ICH], F32, name="opsum",
                                    bufs=3)[0:HD + 1, :])
                    # columns left of the diagonal block are causally
                    # invalid -- skip them in the score matmuls
                    c0 = max(jt - 4 * ic, 0) * P
                    s2 = sp.tile([P, 2 * ICH], F32, name="spsum", bufs=2)
                    jsl = slice(jt * P, (jt + 1) * P)
                    qsl = slice(ic * ICH + c0, (ic + 1) * ICH)
                    # head B's scores land at [ICH : 2*ICH - c0], shifted
                    # left by c0 so the valid spans of A and B abut: the
                    # exp then covers [c0 : 2*ICH - c0] with no dead
                    # middle, trimming ~10us off the scalar engine's
                    # (critical-path) exp stream
                    nc.tensor.matmul(s2[:, c0:ICH], kt_[hp][0:HD, jsl],
                                     qt[hp][0:HD, qsl],
                                     start=True, stop=True)
                    nc.tensor.matmul(s2[:, ICH:2 * ICH - c0],
                                     kt_[hp][HD:P, jsl],
                                     qt[hp][HD:P, qsl],
                                     start=True, stop=True)
                    if ic == 0:
                        # ic0 is DVE-chain-bound: mask the diagonal block
                        # on the PE instead, as an additive -1e5
                        # accumulated onto the scores pre-exp
                        for o in (c0, ICH):
                            nc.tensor.matmul(
                                s2[:, o:o + P], idn_sb[:], neg_sb[:],
                                start=False, stop=True,
                                skip_group_check=True)
                    s2s[g] = s2

                def emit_exp(g):
                    hp, jt = divmod(g, njt)
                    c0 = max(jt - 4 * ic, 0) * P
                    e2 = ap_.tile([P, 2 * ICH], BF16, name="e", bufs=5)
                    s2 = s2s.pop(g)
                    # one activation over the contiguous valid span: the
                    # attention phase is scalar-bound, so every exp
                    # column counts, and the ~410ns of extra per-tile
                    # overhead a split pair of half-activations costs
                    # outweighs the exp->AV latency it would hide
                    nc.scalar.activation(e2[:, c0:2 * ICH - c0],
                                         s2[:, c0:2 * ICH - c0],
                                         AF.Exp)
                    if jt - 4 * ic >= 0 and ic != 0:
                        # zero the diagonal block's upper triangle
                        for o in (c0, ICH):
                            nc.vector.tensor_tensor(
                                e2[:, o:o + P], e2[:, o:o + P],
                                tri_sb[:], op=ALU.mult)
                    e2s[g] = e2

                def emit_av_a(g):
                    hp, jt = divmod(g, njt)
                    c0 = max(jt - 4 * ic, 0) * P
                    nc.tensor.matmul(ops[hp][0][:, c0:],
                                     v3[jt][:, 2 * hp, :],
                                     e2s[g][:, c0:ICH],
                                     start=(jt == 0),
                                     stop=(jt == njt - 1))

                def emit_av_b(g):
                    hp, jt = divmod(g, njt)
                    c0 = max(jt - 4 * ic, 0) * P
                    e2 = e2s.pop(g)
                    nc.tensor.matmul(ops[hp][1][:, c0:],
                                     v3[jt][:, 2 * hp + 1, :],
                                     e2[:, ICH:2 * ICH - c0],
                                     start=(jt == 0),
                                     stop=(jt == njt - 1))

                def norm_pre(ops_, fast_dn=False):
                    # reciprocal of the ones-column denominator row,
                    # broadcast across the head dim; runs while the PE is
                    # still draining the AV tail. (The copy is needed:
                    # reciprocal_approx_fast misreads a PSUM source at a
                    # nonzero base partition.)
                    dn = ap_.tile([1, ICH], F32, name="dn", bufs=4)
                    if fast_dn:
                        nc.scalar.copy(dn[:], ops_[HD:HD + 1, :])
                    else:
                        nc.vector.tensor_copy(dn[:], ops_[HD:HD + 1, :])
                    recip = ap_.tile([1, ICH], F32, name="recip", bufs=4)
                    nc.vector.reciprocal_approx_fast(recip[:], dn[:])
                    return recip

                rbAB = {}

                def norm_a(hp):
                    last = ic == NIC - 1 and hp == MT - 1
                    rc = norm_pre(ops[hp][0], fast_dn=last)
                    rb = ap_.tile([HD, ICH], F32, name="rb", bufs=4)
                    if last:
                        # split broadcasts so the first normalize pieces
                        # land as early as possible for the finale
                        nc.gpsimd.partition_broadcast(rb[:, 0:ICH // 2],
                                                      rc[:, 0:ICH // 2])
                    else:
                        nc.gpsimd.partition_broadcast(rb[:], rc[:])
                    rbAB[hp] = [rb, None, rc]

                def norm_b(hp):
                    last = ic == NIC - 1 and hp == MT - 1
                    rc = norm_pre(ops[hp][1], fast_dn=last)
                    rb = ap_.tile([HD, ICH], F32, name="rb", bufs=4)
                    if last:
                        H2 = ICH // 2
                        nc.gpsimd.partition_broadcast(rb[:, 0:H2],
                                                      rc[:, 0:H2])
                        rcA = rbAB[hp][2]
                        nc.gpsimd.partition_broadcast(
                            rbAB[hp][0][:, H2:ICH], rcA[:, H2:ICH])
                        nc.gpsimd.partition_broadcast(rb[:, H2:ICH],
                                                      rc[:, H2:ICH])
                    else:
                        nc.gpsimd.partition_broadcast(rb[:], rc[:])
                    rbAB[hp][1] = rb

                    def norm_mult(split=False, hp=hp):
                        # normalize straight out of PSUM: in0 is PSUM so
                        # the SBUF base-partition pairing rule doesn't
                        # apply. split=True (very last head pair) emits
                        # 128-column pieces so the finale's k=3 matmuls
                        # unblock progressively instead of all at once.
                        opsA, opsB = ops[hp]
                        rbA, rbB = rbAB[hp][0], rbAB[hp][1]
                        pieces = range(4) if split else (slice(None),)
                        for pc in pieces:
                            csl = (slice(pc * P, (pc + 1) * P)
                                   if isinstance(pc, int) else pc)
                            asl = slice(ic * ICH + (csl.start or 0),
                                        ic * ICH + (csl.stop or ICH))
                            for po, ops_, rb in ((0, opsA, rbA),
                                                 (HD, opsB, rbB)):
                                nc.vector.tensor_tensor(
                                    at[hp][po:po + HD, asl],
                                    ops_[0:HD, csl], rb[:, csl],
                                    op=ALU.mult)

                    pending_mults.append(norm_mult)

                emit_s(0)
                for g in range(1, N + 4):
                    if g < N:
                        emit_s(g)
                        jt = g % njt
                        # coarse filler sites every ~3 j-tiles: the chunk
                        # is scalar-bound, so ~0.3 filler groups per
                        # j-tile of PE work hides under the exp stream.
                        # In the last chunk a small reserve is held back
                        # to cover the AV-B drain + final normalize.
                        site = ((njt == 4 and jt in (2, 3))
                                or (njt > 4 and jt % 3 == 1))
                        if site and gfill and (
                                ic < NIC - 1 or len(gfill) > 2):
                            gfill.pop(0)()
                    # normalize multiplies appended last iteration run
                    # now: behind the masks already queued on the DVE,
                    # but ahead of the opsum slot's next writer
                    while pending_mults:
                        pending_mults.pop()()
                    if g <= N:
                        emit_exp(g - 1)
                        emit_av_a(g - 1)
                        if g % njt == 0:
                            norm_a(g // njt - 1)
                    if g >= 4:
                        emit_av_b(g - 4)
                        if (g - 3) % njt == 0:
                            norm_b((g - 3) // njt - 1)

                while pending_mults:
                    pending_mults.pop()(split=(ic == NIC - 1))

            def emit_out_group(mt, nch2, pso=None, copy_eng='vector',
                               dma_eng='sync'):
                if pso is None:
                    pso = sp.tile([P, 512], F32, name="misc", bufs=1)
                for k in range(MT):
                    nc.tensor.matmul(
                        pso[:], at[k][:, mt * P:(mt + 1) * P],
                        wo_sb[:, k, nch2 * 512:(nch2 + 1) * 512],
                        start=(k == 0), stop=(k == MT - 1))
                ob = obp.tile([P, 512], F32, name="ob")
                if copy_eng == 'scalar':
                    nc.scalar.copy(ob[:], pso[:])
                else:
                    nc.vector.tensor_copy(ob[:], pso[:])
                getattr(nc, dma_eng).dma_start(
                    out_d[mt * P:(mt + 1) * P,
                          nch2 * 512:(nch2 + 1) * 512], ob[:])

            # global filler FIFO, ordered by deadline: V tiles 4..7 feed
            # chunk 1, the nch=2 half of Q^T/K^T feeds chunk 2, V 8..11
            # feed chunk 2, the nch=3 half feeds chunk 3, V 12..15 feed
            # chunk 3's own late j-tiles, and the out-projection groups of
            # finished chunks can run any time after their chunk's
            # normalize. ~68 sites across the four chunks drain 52 groups.
            gfill = []
            # mt3's first-half projections: ic0 only reads qt/kt cols
            # 0:512, so only the chunk-0 pieces gate ic0's last head pair
            gfill += [(lambda w=w, n=n: a1h1_group(3, w, n))
                      for n in range(2) for w in ('q', 'k')]
            gfill += [(lambda tt=tt: emit_v_tile(tt)) for tt in range(4, 8)]
            gfill += [(lambda mt=mt, w=w: a1h1_group(mt, w, 2))
                      for mt in range(MT) for w in ('q', 'k')]
            gfill += [(lambda tt=tt: emit_v_tile(tt)) for tt in range(8, 12)]
            gfill += [(lambda mt=mt, w=w: a1h1_group(mt, w, 3))
                      for mt in range(MT) for w in ('q', 'k')]
            gfill += [(lambda tt=tt: emit_v_tile(tt)) for tt in range(12, 16)]
            gfill += [(lambda mt=mt, n=n: emit_out_group(mt, n))
                      for mt in range(12) for n in range(2)]
            for ic in range(NIC):
                emit_attn_chunk(ic)
            # drain the reserve: its matmuls land between the last AV-B
            # drain and the finale waves, covering the normalize chain
            while gfill:
                gfill.pop(0)()

            # final out-projection, mt 12..15 x both halves. The (12,*)
            # and (13,*) groups run k-major on the four spsum bank halves
            # (free once the last exps retire, and with no WAR on any
            # copy) so the PE streams straight through the last head
            # pair's normalize chain; their k=3 matmuls follow in piece
            # order, matching the column-split normalize. (14,0) sits on
            # misc (WAR on the last in-loop filler's copy clears behind
            # the spsum waves), (14,1) on the opsum slot freed by hp2's
            # normalize, and (15,*) on the final head pair's own opsum
            # slots once its normalize mults have read them. Copies
            # alternate scalar/vector per stop order, and the DMAs are
            # spread over the sync, gpsimd and tensor DGE queues so the
            # ~600ns issue cost is paid three lanes wide.
            sps = []
            for _ in range(2):
                t = sp.tile([P, 2 * ICH], F32, name="spsum", bufs=2)
                sps += [t[:, 0:512], t[:, 512:1024]]
            wave = [(12, 0), (12, 1), (13, 0), (13, 1)]
            for k in range(MT - 1):
                for g, (mt, nch2) in enumerate(wave):
                    nc.tensor.matmul(
                        sps[g], at[k][:, mt * P:(mt + 1) * P],
                        wo_sb[:, k, nch2 * 512:(nch2 + 1) * 512],
                        start=(k == 0), stop=False)
            for g, (mt, nch2) in enumerate(wave):
                nc.tensor.matmul(
                    sps[g], at[MT - 1][:, mt * P:(mt + 1) * P],
                    wo_sb[:, MT - 1, nch2 * 512:(nch2 + 1) * 512],
                    start=False, stop=True)
            omisc = sp.tile([P, 512], F32, name="misc", bufs=1)
            opst = [sp.tile([P, ICH], F32, name="opsum", bufs=3)
                    for _ in range(3)]
            tailg = [((14, 0), omisc), ((14, 1), opst[0]),
                     ((15, 0), opst[1]), ((15, 1), opst[2])]
            for (mt, nch2), pso in tailg:
                for k in range(MT):
                    nc.tensor.matmul(
                        pso[:], at[k][:, mt * P:(mt + 1) * P],
                        wo_sb[:, k, nch2 * 512:(nch2 + 1) * 512],
                        start=(k == 0), stop=(k == MT - 1))
            outs = [((12, 0), sps[0], 'scalar', 'sync'),
                    ((12, 1), sps[1], 'vector', 'sync'),
                    ((13, 0), sps[2], 'scalar', 'sync'),
                    ((13, 1), sps[3], 'vector', 'sync'),
                    ((14, 0), omisc[:], 'scalar', 'sync'),
                    ((14, 1), opst[0][:], 'vector', 'sync'),
                    ((15, 0), opst[1][:], 'scalar', 'sync'),
                    ((15, 1), opst[2][:], 'vector', 'sync')]
            for (mt, nch2), pso, ce, de in outs:
                ob = obp.tile([P, 512], F32, name="ob")
                if ce == 'scalar':
                    nc.scalar.copy(ob[:], pso)
                else:
                    nc.vector.tensor_copy(ob[:], pso)
                getattr(nc, de).dma_start(
                    out_d[mt * P:(mt + 1) * P,
                          nch2 * 512:(nch2 + 1) * 512], ob[:])

    nc.compile()
    return nc


def _get_program():
    global _PROGRAM
    if _PROGRAM is None:
        _install_ntff_hook()
        _PROGRAM = _build_program()
    return _PROGRAM


def _make_masks():
    """Multiplicative upper-triangle zero mask [128, 128] for the diagonal
    128x128 block of each S^T tile: entry (j, i) = 1 if j <= i else 0."""
    j = np.arange(P)[:, None]
    i = np.arange(P)[None, :]
    return (j <= i).astype(np.float32)


def make_in_maps(x, Wq, bq, Wk, bk, Wv, bv, Wo, bo):
    import ml_dtypes
    bf16 = ml_dtypes.bfloat16

    def sbl(a, k):
        """[k*128, n] -> SBUF layout [128, k*n] (partition-major runs)."""
        n = a.shape[1]
        return np.ascontiguousarray(
            a.reshape(k, P, n).transpose(1, 0, 2).reshape(P, k * n)
        ).astype(bf16)

    # bk is not shipped to the device: for a fixed query the q.bk and
    # bq.bk score terms are constant across keys and cancel in softmax.
    masks = _make_masks()
    in_maps = []
    for c in range(8):
        b, hg = c // 2, c % 2
        sl = slice(hg * QD, (hg + 1) * QD)
        in_maps.append({
            "xT": sbl(np.ascontiguousarray(x[b].T), KT),
            "wq": sbl(Wq[:, sl] * SCALE, KT),
            "wk": sbl(Wk[:, sl], KT),
            "wv": sbl(Wv[:, sl], KT),
            "wo": sbl(Wo[sl, :], MT),
            "bq": np.ascontiguousarray((bq[sl] * SCALE).reshape(MT, P).T),
            "msk": masks.astype(bf16),
            "idn": np.eye(P, dtype=np.float32).astype(bf16),
            "neg": ((1.0 - masks) * -100000.0).astype(bf16),
        })
    return in_maps


def run(inputs, trace=False):
    from concourse.bass_utils import run_bass_kernel_spmd

    nc = _get_program()
    in_maps = make_in_maps(**inputs)
    res = run_bass_kernel_spmd(nc, in_maps, list(range(8)), trace=trace)
    # softmax rows sum to 1, so the V bias adds bv to every attention
    # output exactly; fold bv @ Wo into the host-side output bias
    bo_eff = inputs["bo"] + inputs["bv"].astype(np.float64) @ \
        inputs["Wo"].astype(np.float64)
    bo_eff = bo_eff.astype(np.float32)
    out = np.empty((B, T, D), dtype=np.float32)
    for b in range(B):
        out[b] = res.results[2 * b]["out"] + res.results[2 * b + 1]["out"] \
            + bo_eff
    return out, res


def kernel(**inputs):
    inputs = {k: np.asarray(v) for k, v in inputs.items()}
    out, _ = run(inputs)
    return out



# revision 37
# speedup vs baseline: 1.0162x; 1.0162x over previous
"""Causal self-attention (B=4, T=2048, D=1024, H=16, hd=64) on 8 trn2 NeuronCores.

Sharding: data parallel over batch (4) x tensor parallel over heads (2 groups
of 8). Core c handles batch c//2 and heads (c%2)*8 .. (c%2)*8+8.
Wq/Wk/Wv are column-parallel by head group, Wo row-parallel; the pair of
cores sharing a batch produce partial outputs that are summed on the host.

On-device layout (per core) is fully "transposed": projections produce
Q^T, K^T [512, 2048] and V [2048, 512], scores are computed as
S^T = K Q^T (j=key on partitions, i=query on free dim), softmax uses
exp without max subtraction (scores are O(6) here), the denominator
comes for free from a ones-column appended to V, and attention output
O^T [hd, T] feeds the row-parallel out-projection directly as lhsT.

The V bias never touches the device: softmax rows sum to 1, so
attn @ (V + bv) = attn @ V + bv, and bv @ Wo folds into the host-side
output bias.

Head pairs share one [128, 1024] exp; their S^T matmuls row-pack onto
the PE concurrently (partition offsets 0/64). Score matmuls and exps are
trimmed to causally-valid columns. The per-chunk emission is
software-pipelined (S of tile jt+1 ahead of AV of tile jt in the PE
stream) so the PE never waits on the scalar engine's exp; AV-B lags
AV-A by two j-tiles so the psum-slot WAR against the previous head
pair's normalize never stalls the PE.
"""

import contextlib
import ctypes
import sys
import types

import numpy as np

B, T, D = 4, 2048, 1024
H_TOT, HD = 16, 64
SCALE = HD ** -0.5
P = 128
NH = 8            # heads per core
QD = NH * HD      # 512, projected dim per core
KT = D // P       # 8 contraction tiles for projections
MT = QD // P      # 4 qdim tiles
TT = T // P       # 16 token tiles
ACH = 512         # phase-A1 token chunk (Q/K); PSUM bank caps matmul N at 512
NACH = T // ACH   # 4
ICH = 512         # attention query chunk
NIC = T // ICH    # 4

_PROGRAM = None  # compiled program cache — build once per process


def _install_ntff_hook():
    """antenv.axon_hooks is missing in this image; recreate it so
    run_bass_kernel_spmd(trace=True) can profile. Harmless if unused."""
    if "antenv.axon_hooks" in sys.modules:
        return
    try:
        import antenv
    except ImportError:
        return
    mod = types.ModuleType("antenv.axon_hooks")
    _hook = [None]
    mod.set_axon_ntff_profile_hook = lambda h: _hook.__setitem__(0, h)
    mod.get_axon_ntff_profile_hook = lambda: _hook[0]
    antenv.axon_hooks = mod
    sys.modules["antenv.axon_hooks"] = mod
    try:
        lib = ctypes.CDLL("/opt/axon/libaxon_pjrt.so")
        if not hasattr(lib, "axon_start_nrt_profile"):
            return
        lib.axon_start_nrt_profile.argtypes = [
            ctypes.POINTER(ctypes.c_int64), ctypes.c_size_t]
        lib.axon_start_nrt_profile.restype = ctypes.c_int64
        lib.axon_stop_nrt_profile.argtypes = [ctypes.c_char_p]
        lib.axon_stop_nrt_profile.restype = ctypes.c_int64

        @contextlib.contextmanager
        def _hookfn(output_dir, device_ids):
            import jax
            jax.devices()
            if device_ids:
                ids = (ctypes.c_int64 * len(device_ids))(*device_ids)
                rc = lib.axon_start_nrt_profile(ids, len(device_ids))
            else:
                rc = lib.axon_start_nrt_profile(None, 0)
            if rc != 0:
                raise RuntimeError(f"axon_start_nrt_profile rc={rc}")
            try:
                yield
            finally:
                n = lib.axon_stop_nrt_profile(str(output_dir).encode())
                print(f"profile: {n} file(s) written to {output_dir}")

        mod.set_axon_ntff_profile_hook(_hookfn)
    except OSError:
        pass


def _build_program():
    from contextlib import ExitStack

    import concourse.tile as tile
    from concourse import bacc, mybir

    F32 = mybir.dt.float32
    BF16 = mybir.dt.bfloat16
    AF = mybir.ActivationFunctionType
    ALU = mybir.AluOpType

    nc = bacc.Bacc("TRN2", target_bir_lowering=False, debug=False,
                   num_devices=8)

    # all tensor inputs arrive pre-arranged in SBUF layout [128, k, n]
    # (host does the transpose) so every DMA is long contiguous runs
    xT_d = nc.dram_tensor("xT", [P, KT * T], BF16, kind="ExternalInput").ap()
    wq_d = nc.dram_tensor("wq", [P, KT * QD], BF16, kind="ExternalInput").ap()
    wk_d = nc.dram_tensor("wk", [P, KT * QD], BF16, kind="ExternalInput").ap()
    wv_d = nc.dram_tensor("wv", [P, KT * QD], BF16, kind="ExternalInput").ap()
    wo_d = nc.dram_tensor("wo", [P, MT * D], BF16, kind="ExternalInput").ap()
    bq_d = nc.dram_tensor("bq", [P, MT], F32, kind="ExternalInput").ap()
    msk_d = nc.dram_tensor("msk", [P, P], BF16, kind="ExternalInput").ap()
    idn_d = nc.dram_tensor("idn", [P, P], BF16, kind="ExternalInput").ap()
    neg_d = nc.dram_tensor("neg", [P, P], BF16, kind="ExternalInput").ap()
    out_d = nc.dram_tensor("out", [T, D], F32, kind="ExternalOutput").ap()

    xT_k = xT_d.rearrange("p (k t) -> p k t", k=KT)      # [128, 8, 2048]
    wq_k = wq_d.rearrange("p (k m) -> p k m", k=KT)      # [128, 8, 512]
    wk_k = wk_d.rearrange("p (k m) -> p k m", k=KT)
    wv_k = wv_d.rearrange("p (k m) -> p k m", k=KT)
    wo_k = wo_d.rearrange("p (k e) -> p k e", k=MT)      # [128, 4, 1024]

    with tile.TileContext(nc) as tc, ExitStack() as ctx:
        persist = ctx.enter_context(tc.tile_pool(name="persist", bufs=1))

        qt = [persist.tile([P, T], BF16, name=f"qt{i}") for i in range(MT)]
        kt_ = [persist.tile([P, T], BF16, name=f"kt{i}") for i in range(MT)]
        v3 = [persist.tile([P, NH, HD + 1], BF16, name=f"v3_{i}")
              for i in range(TT)]
        at = [persist.tile([P, T], BF16, name=f"at{i}") for i in range(MT)]
        xt_all = persist.tile([P, KT, T], BF16, name="xt")

        wq_sb = persist.tile([P, KT, QD], BF16, name="wq")
        wk_sb = persist.tile([P, KT, QD], BF16, name="wk")
        bq_sb = persist.tile([P, MT], F32, name="bq")
        tri_sb = persist.tile([P, P], BF16, name="tri")
        idn_sb = persist.tile([P, P], BF16, name="idn")
        neg_sb = persist.tile([P, P], BF16, name="neg")
        wv_sb = persist.tile([P, KT, QD], BF16, name="wv")
        wo_sb = persist.tile([P, MT, D], BF16, name="wo")

        # startup DMAs are issued from FOUR engine DGE queues in parallel
        # (sync/scalar/vector/gpsimd are all idle during the preamble), so
        # the first matmul's inputs and the k=1..7 tiles phase A1 streams
        # through all arrive several us earlier than a single-queue issue
        # order could deliver them. Per queue, DMAs are ordered by the time
        # phase A1 consumes them.
        # weight DMAs issue from the scalar engine's hardware DGE queue,
        # in parallel with the sync queue streaming x^T: phase A1's first
        # groups stop starving on single-queue issue order. (gpsimd would
        # be a THIRD queue but is software DGE -- ~100x slower transfers.)
        nc.scalar.dma_start(wq_sb[:, 0, 0:P], wq_k[:, 0, 0:P])
        nc.sync.dma_start(xt_all[:, 0, 0:ACH], xT_k[:, 0, 0:ACH])
        nc.scalar.dma_start(wq_sb[:, 0, P:QD], wq_k[:, 0, P:QD])
        nc.sync.dma_start(xt_all[:, 0, ACH:T // 2], xT_k[:, 0, ACH:T // 2])
        for k in range(1, KT):
            nc.scalar.dma_start(wq_sb[:, k, :], wq_k[:, k, :])
            nc.sync.dma_start(xt_all[:, k, 0:T // 2], xT_k[:, k, 0:T // 2])
        for k in range(KT):
            nc.scalar.dma_start(wk_sb[:, k, :], wk_k[:, k, :])
        nc.scalar.dma_start(bq_sb[:], bq_d)
        nc.sync.dma_start(idn_sb[:], idn_d)
        nc.sync.dma_start(neg_sb[:], neg_d)
        for k in range(KT):
            nc.sync.dma_start(xt_all[:, k, T // 2:T], xT_k[:, k, T // 2:T])
        nc.sync.dma_start(tri_sb[:], msk_d)
        nc.sync.dma_start(wv_sb[:], wv_k)
        nc.sync.dma_start(wo_sb[:], wo_k)
        for tt in range(TT):
            nc.vector.memset(v3[tt][:, :, HD:HD + 1], 1.0)

        # ---- phase A1: Q^T, K^T projections --------------------------------
        with tc.tile_pool(name="pjps1", bufs=1, space="PSUM") as pjp:
            # chunk-inner so each weight tile is loaded into the PE once
            # and reused for two 512-column chunks; two half-passes so the
            # first matmuls only wait on the first half of the xT DMA
            # bk is dropped entirely: for a fixed query, the q.bk and bq.bk
            # score terms are constant across keys, so they cancel in
            # softmax; only the Q bias survives. K^T's psum->SBUF moves go
            # on the (otherwise idle in A1) scalar engine so the DVE only
            # carries Q's bias-adds and the two move queues drain in
            # parallel at the A1->attention transition.
            for half in range(1):
                for mt in range(MT):
                    for w_sb, dst, b_sb in ((wq_sb, qt, bq_sb),
                                            (wk_sb, kt_, None)):
                        ps = [pjp.tile([P, ACH], F32, name="pj", bufs=8)
                              for _ in range(2)]
                        for k in range(KT):
                            for i, nch in enumerate((2 * half, 2 * half + 1)):
                                nc.tensor.matmul(
                                    ps[i][:],
                                    w_sb[:, k, mt * P:(mt + 1) * P],
                                    xt_all[:, k, nch * ACH:(nch + 1) * ACH],
                                    start=(k == 0), stop=(k == KT - 1))
                        for i, nch in enumerate((2 * half, 2 * half + 1)):
                            csl = slice(nch * ACH, (nch + 1) * ACH)
                            if b_sb is None:
                                nc.scalar.copy(dst[mt][:, csl], ps[i][:])
                            else:
                                nc.vector.tensor_scalar_add(dst[mt][:, csl],
                                                            ps[i][:],
                                                            b_sb[:, mt:mt + 1])
            # first V tiles inside the A1 psum epoch: their psum slot's
            # previous user retired 8 allocations ago, so no pool-close
            # barrier stalls the PE at the A1->attention transition
            for tt in range(4):
                psv = pjp.tile([P, ACH], F32, name="pj", bufs=8)
                for k in range(KT):
                    nc.tensor.matmul(
                        psv[:], xt_all[:, k, tt * P:(tt + 1) * P],
                        wv_sb[:, k, :], start=(k == 0), stop=(k == KT - 1))
                nc.scalar.copy(
                    v3[tt][:, :, 0:HD],
                    psv[:].rearrange("p (h d) -> p h d", d=HD))

        # ---- phases A2/B/C interleaved per query chunk ---------------------
        with tc.tile_pool(name="attnsb", bufs=1) as ap_, \
             tc.tile_pool(name="obp", bufs=8) as obp, \
             tc.tile_pool(name="attnps", bufs=1, space="PSUM") as sp:

            def emit_v_tile(tt):
                psv = sp.tile([P, QD], F32, name="misc", bufs=1)
                for k in range(KT):
                    nc.tensor.matmul(
                        psv[:], xt_all[:, k, tt * P:(tt + 1) * P],
                        wv_sb[:, k, :], start=(k == 0), stop=(k == KT - 1))
                nc.vector.tensor_copy(
                    v3[tt][:, :, 0:HD],
                    psv[:].rearrange("p (h d) -> p h d", d=HD))

            def a1h1_group(mt, w, nch):
                # second-half Q^T/K^T projection (token cols 1024:2048),
                # hoisted out of the serial phase-A1 and run as attention
                # fillers: the attention phase is scalar(exp)-bound, so
                # this PE work hides entirely under the exp stream. The
                # psum->SBUF moves go on the DVE (the scalar engine is the
                # critical path here, unlike in phase A1).
                ps = sp.tile([P, ACH], F32, name="misc", bufs=1)
                w_sb = wq_sb if w == 'q' else wk_sb
                for k in range(KT):
                    nc.tensor.matmul(
                        ps[:], w_sb[:, k, mt * P:(mt + 1) * P],
                        xt_all[:, k, nch * ACH:(nch + 1) * ACH],
                        start=(k == 0), stop=(k == KT - 1))
                csl = slice(nch * ACH, (nch + 1) * ACH)
                if w == 'q':
                    nc.vector.tensor_scalar_add(qt[mt][:, csl], ps[:],
                                                bq_sb[:, mt:mt + 1])
                else:
                    nc.vector.tensor_copy(kt_[mt][:, csl], ps[:])

            def emit_attn_chunk(ic):
                """Attention for query chunk ic as ONE flat software
                pipeline over all (head-pair, j-tile) pairs: S of tile
                g+1 is emitted before AV of tile g so exp latency is
                hidden, and the pipeline flows STRAIGHT ACROSS head-pair
                boundaries -- the next pair's scores enter the PE stream
                while the previous pair's AV-B tail drains, so the scalar
                engine (the critical path) never starves at a boundary.
                AV-B lags AV-A by four tiles so the opsum-slot WAR
                against the previous head pair's normalize clears before
                the PE reaches the first AV-B. Each normalize's
                reciprocal+broadcast are emitted as soon as its ops
                accumulation stops; its multiplies are deferred exactly
                one tile so they queue on the DVE behind the next tile's
                masks (any earlier and they would stall the mask feed,
                any later and the opsum slot's next user would outrun
                them)."""
                njt = 4 * ic + 4
                N = MT * njt
                pending_mults = []
                ops = {}
                s2s, e2s = {}, {}

                def emit_s(g):
                    hp, jt = divmod(g, njt)
                    if jt == 0:
                        # allocated full-bank so the final out-projection
                        # waves can reuse freed opsum slots with an
                        # identical tag shape
                        ops[hp] = (
                            sp.tile([P, ICH], F32, name="opsum",
                                    bufs=3)[0:HD + 1, :],
                            sp.tile([P, ICH], F32, name="opsum",
                                    bufs=3)[0:HD + 1, :])
                    # columns left of the diagonal block are causally
                    # invalid -- skip them in the score matmuls
                    c0 = max(jt - 4 * ic, 0) * P
                    s2 = sp.tile([P, 2 * ICH], F32, name="spsum", bufs=2)
                    jsl = slice(jt * P, (jt + 1) * P)
                    qsl = slice(ic * ICH + c0, (ic + 1) * ICH)
                    # head B's scores land at [ICH : 2*ICH - c0], shifted
                    # left by c0 so the valid spans of A and B abut: the
                    # exp then covers [c0 : 2*ICH - c0] with no dead
                    # middle, trimming ~10us off the scalar engine's
                    # (critical-path) exp stream
                    nc.tensor.matmul(s2[:, c0:ICH], kt_[hp][0:HD, jsl],
                                     qt[hp][0:HD, qsl],
                                     start=True, stop=True)
                    nc.tensor.matmul(s2[:, ICH:2 * ICH - c0],
                                     kt_[hp][HD:P, jsl],
                                     qt[hp][HD:P, qsl],
                                     start=True, stop=True)
                    if ic == 0:
                        # ic0 is DVE-chain-bound: mask the diagonal block
                        # on the PE instead, as an additive -1e5
                        # accumulated onto the scores pre-exp
                        for o in (c0, ICH):
                            nc.tensor.matmul(
                                s2[:, o:o + P], idn_sb[:], neg_sb[:],
                                start=False, stop=True,
                                skip_group_check=True)
                    s2s[g] = s2

                def emit_exp(g):
                    hp, jt = divmod(g, njt)
                    c0 = max(jt - 4 * ic, 0) * P
                    e2 = ap_.tile([P, 2 * ICH], BF16, name="e", bufs=5)
                    s2 = s2s.pop(g)
                    # one activation over the contiguous valid span: the
                    # attention phase is scalar-bound, so every exp
                    # column counts, and the ~410ns of extra per-tile
                    # overhead a split pair of half-activations costs
                    # outweighs the exp->AV latency it would hide
                    nc.scalar.activation(e2[:, c0:2 * ICH - c0],
                                         s2[:, c0:2 * ICH - c0],
                                         AF.Exp)
                    if jt - 4 * ic >= 0 and ic != 0:
                        # zero the diagonal block's upper triangle
                        for o in (c0, ICH):
                            nc.vector.tensor_tensor(
                                e2[:, o:o + P], e2[:, o:o + P],
                                tri_sb[:], op=ALU.mult)
                    e2s[g] = e2

                def emit_av_a(g):
                    hp, jt = divmod(g, njt)
                    c0 = max(jt - 4 * ic, 0) * P
                    nc.tensor.matmul(ops[hp][0][:, c0:],
                                     v3[jt][:, 2 * hp, :],
                                     e2s[g][:, c0:ICH],
                                     start=(jt == 0),
                                     stop=(jt == njt - 1))

                def emit_av_b(g):
                    hp, jt = divmod(g, njt)
                    c0 = max(jt - 4 * ic, 0) * P
                    e2 = e2s.pop(g)
                    nc.tensor.matmul(ops[hp][1][:, c0:],
                                     v3[jt][:, 2 * hp + 1, :],
                                     e2[:, ICH:2 * ICH - c0],
                                     start=(jt == 0),
                                     stop=(jt == njt - 1))

                def norm_pre(ops_, fast_dn=False):
                    # reciprocal of the ones-column denominator row,
                    # broadcast across the head dim; runs while the PE is
                    # still draining the AV tail. (The copy is needed:
                    # reciprocal_approx_fast misreads a PSUM source at a
                    # nonzero base partition.)
                    dn = ap_.tile([1, ICH], F32, name="dn", bufs=4)
                    if fast_dn:
                        nc.scalar.copy(dn[:], ops_[HD:HD + 1, :])
                    else:
                        nc.vector.tensor_copy(dn[:], ops_[HD:HD + 1, :])
                    recip = ap_.tile([1, ICH], F32, name="recip", bufs=4)
                    nc.vector.reciprocal_approx_fast(recip[:], dn[:])
                    return recip

                rbAB = {}

                def norm_a(hp):
                    last = ic == NIC - 1 and hp == MT - 1
                    rc = norm_pre(ops[hp][0], fast_dn=last)
                    rb = ap_.tile([HD, ICH], F32, name="rb", bufs=4)
                    if last:
                        # split broadcasts so the first normalize pieces
                        # land as early as possible for the finale
                        nc.gpsimd.partition_broadcast(rb[:, 0:ICH // 2],
                                                      rc[:, 0:ICH // 2])
                    else:
                        nc.gpsimd.partition_broadcast(rb[:], rc[:])
                    rbAB[hp] = [rb, None, rc]

                def norm_b(hp):
                    last = ic == NIC - 1 and hp == MT - 1
                    rc = norm_pre(ops[hp][1], fast_dn=last)
                    rb = ap_.tile([HD, ICH], F32, name="rb", bufs=4)
                    if last:
                        H2 = ICH // 2
                        nc.gpsimd.partition_broadcast(rb[:, 0:H2],
                                                      rc[:, 0:H2])
                        rcA = rbAB[hp][2]
                        nc.gpsimd.partition_broadcast(
                            rbAB[hp][0][:, H2:ICH], rcA[:, H2:ICH])
                        nc.gpsimd.partition_broadcast(rb[:, H2:ICH],
                                                      rc[:, H2:ICH])
                    else:
                        nc.gpsimd.partition_broadcast(rb[:], rc[:])
                    rbAB[hp][1] = rb

                    def norm_mult(split=False, hp=hp):
                        # normalize straight out of PSUM: in0 is PSUM so
                        # the SBUF base-partition pairing rule doesn't
                        # apply. split=True (very last head pair) emits
                        # 128-column pieces so the finale's k=3 matmuls
                        # unblock progressively instead of all at once.
                        opsA, opsB = ops[hp]
                        rbA, rbB = rbAB[hp][0], rbAB[hp][1]
                        pieces = range(4) if split else (slice(None),)
                        for pc in pieces:
                            csl = (slice(pc * P, (pc + 1) * P)
                                   if isinstance(pc, int) else pc)
                            asl = slice(ic * ICH + (csl.start or 0),
                                        ic * ICH + (csl.stop or ICH))
                            for po, ops_, rb in ((0, opsA, rbA),
                                                 (HD, opsB, rbB)):
                                nc.vector.tensor_tensor(
                                    at[hp][po:po + HD, asl],
                                    ops_[0:HD, csl], rb[:, csl],
                                    op=ALU.mult)

                    pending_mults.append(norm_mult)

                emit_s(0)
                for g in range(1, N + 4):
                    if g < N:
                        emit_s(g)
                        jt = g % njt
                        # coarse filler sites every ~3 j-tiles: the chunk
                        # is scalar-bound, so ~0.3 filler groups per
                        # j-tile of PE work hides under the exp stream.
                        # In the last chunk a small reserve is held back
                        # to cover the AV-B drain + final normalize.
                        site = ((njt == 4 and jt == 2)
                                or (njt > 4 and jt % 3 == 1))
                        if site and gfill and (
                                ic < NIC - 1 or len(gfill) > 2):
                            gfill.pop(0)()
                    # normalize multiplies appended last iteration run
                    # now: behind the masks already queued on the DVE,
                    # but ahead of the opsum slot's next writer
                    while pending_mults:
                        pending_mults.pop()()
                    if g <= N:
                        emit_exp(g - 1)
                        emit_av_a(g - 1)
                        if g % njt == 0:
                            norm_a(g // njt - 1)
                    if g >= 4:
                        emit_av_b(g - 4)
                        if (g - 3) % njt == 0:
                            norm_b((g - 3) // njt - 1)

                while pending_mults:
                    pending_mults.pop()(split=(ic == NIC - 1))

            def emit_out_group(mt, nch2, pso=None, copy_eng='vector',
                               dma_eng='sync'):
                if pso is None:
                    pso = sp.tile([P, 512], F32, name="misc", bufs=1)
                for k in range(MT):
                    nc.tensor.matmul(
                        pso[:], at[k][:, mt * P:(mt + 1) * P],
                        wo_sb[:, k, nch2 * 512:(nch2 + 1) * 512],
                        start=(k == 0), stop=(k == MT - 1))
                ob = obp.tile([P, 512], F32, name="ob")
                if copy_eng == 'scalar':
                    nc.scalar.copy(ob[:], pso[:])
                else:
                    nc.vector.tensor_copy(ob[:], pso[:])
                getattr(nc, dma_eng).dma_start(
                    out_d[mt * P:(mt + 1) * P,
                          nch2 * 512:(nch2 + 1) * 512], ob[:])

            # global filler FIFO, ordered by deadline: V tiles 4..7 feed
            # chunk 1, the nch=2 half of Q^T/K^T feeds chunk 2, V 8..11
            # feed chunk 2, the nch=3 half feeds chunk 3, V 12..15 feed
            # chunk 3's own late j-tiles, and the out-projection groups of
            # finished chunks can run any time after their chunk's
            # normalize. ~68 sites across the four chunks drain 52 groups.
            gfill = []
            gfill += [(lambda tt=tt: emit_v_tile(tt)) for tt in range(4, 8)]
            gfill += [(lambda mt=mt, w=w: a1h1_group(mt, w, 2))
                      for mt in range(MT) for w in ('q', 'k')]
            gfill += [(lambda tt=tt: emit_v_tile(tt)) for tt in range(8, 12)]
            gfill += [(lambda mt=mt, w=w: a1h1_group(mt, w, 3))
                      for mt in range(MT) for w in ('q', 'k')]
            gfill += [(lambda tt=tt: emit_v_tile(tt)) for tt in range(12, 16)]
            gfill += [(lambda mt=mt, n=n: emit_out_group(mt, n))
                      for mt in range(12) for n in range(2)]
            for ic in range(NIC):
                emit_attn_chunk(ic)
            # drain the reserve: its matmuls land between the last AV-B
            # drain and the finale waves, covering the normalize chain
            while gfill:
                gfill.pop(0)()

            # final out-projection, mt 12..15 x both halves. The (12,*)
            # and (13,*) groups run k-major on the four spsum bank halves
            # (free once the last exps retire, and with no WAR on any
            # copy) so the PE streams straight through the last head
            # pair's normalize chain; their k=3 matmuls follow in piece
            # order, matching the column-split normalize. (14,0) sits on
            # misc (WAR on the last in-loop filler's copy clears behind
            # the spsum waves), (14,1) on the opsum slot freed by hp2's
            # normalize, and (15,*) on the final head pair's own opsum
            # slots once its normalize mults have read them. Copies
            # alternate scalar/vector per stop order, and the DMAs are
            # spread over the sync, gpsimd and tensor DGE queues so the
            # ~600ns issue cost is paid three lanes wide.
            sps = []
            for _ in range(2):
                t = sp.tile([P, 2 * ICH], F32, name="spsum", bufs=2)
                sps += [t[:, 0:512], t[:, 512:1024]]
            wave = [(12, 0), (12, 1), (13, 0), (13, 1)]
            for k in range(MT - 1):
                for g, (mt, nch2) in enumerate(wave):
                    nc.tensor.matmul(
                        sps[g], at[k][:, mt * P:(mt + 1) * P],
                        wo_sb[:, k, nch2 * 512:(nch2 + 1) * 512],
                        start=(k == 0), stop=False)
            for g, (mt, nch2) in enumerate(wave):
                nc.tensor.matmul(
                    sps[g], at[MT - 1][:, mt * P:(mt + 1) * P],
                    wo_sb[:, MT - 1, nch2 * 512:(nch2 + 1) * 512],
                    start=False, stop=True)
            omisc = sp.tile([P, 512], F32, name="misc", bufs=1)
            opst = [sp.tile([P, ICH], F32, name="opsum", bufs=3)
                    for _ in range(3)]
            tailg = [((14, 0), omisc), ((14, 1), opst[0]),
                     ((15, 0), opst[1]), ((15, 1), opst[2])]
            for (mt, nch2), pso in tailg:
                for k in range(MT):
                    nc.tensor.matmul(
                        pso[:], at[k][:, mt * P:(mt + 1) * P],
                        wo_sb[:, k, nch2 * 512:(nch2 + 1) * 512],
                        start=(k == 0), stop=(k == MT - 1))
            outs = [((12, 0), sps[0], 'scalar', 'sync'),
                    ((12, 1), sps[1], 'vector', 'sync'),
                    ((13, 0), sps[2], 'scalar', 'sync'),
                    ((13, 1), sps[3], 'vector', 'sync'),
                    ((14, 0), omisc[:], 'scalar', 'sync'),
                    ((14, 1), opst[0][:], 'vector', 'sync'),
                    ((15, 0), opst[1][:], 'scalar', 'sync'),
                    ((15, 1), opst[2][:], 'vector', 'sync')]
            for (mt, nch2), pso, ce, de in outs:
                ob = obp.tile([P, 512], F32, name="ob")
                if ce == 'scalar':
                    nc.scalar.copy(ob[:], pso)
                else:
                    nc.vector.tensor_copy(ob[:], pso)
                getattr(nc, de).dma_start(
                    out_d[mt * P:(mt + 1) * P,
                          nch2 * 512:(nch2 + 1) * 512], ob[:])

    nc.compile()
    return nc


def _get_program():
    global _PROGRAM
    if _PROGRAM is None:
        _install_ntff_hook()
        _PROGRAM = _build_program()
    return _PROGRAM


def _make_masks():
    """Multiplicative upper-triangle zero mask [128, 128] for the diagonal
    128x128 block of each S^T tile: entry (j, i) = 1 if j <= i else 0."""
    j = np.arange(P)[:, None]
    i = np.arange(P)[None, :]
    return (j <= i).astype(np.float32)


def make_in_maps(x, Wq, bq, Wk, bk, Wv, bv, Wo, bo):
    import ml_dtypes
    bf16 = ml_dtypes.bfloat16

    def sbl(a, k):
        """[k*128, n] -> SBUF layout [128, k*n] (partition-major runs)."""
        n = a.shape[1]
        return np.ascontiguousarray(
            a.reshape(k, P, n).transpose(1, 0, 2).reshape(P, k * n)
        ).astype(bf16)

    # bk is not shipped to the device: for a fixed query the q.bk and
    # bq.bk score terms are constant across keys and cancel in softmax.
    masks = _make_masks()
    in_maps = []
    for c in range(8):
        b, hg = c // 2, c % 2
        sl = slice(hg * QD, (hg + 1) * QD)
        in_maps.append({
            "xT": sbl(np.ascontiguousarray(x[b].T), KT),
            "wq": sbl(Wq[:, sl] * SCALE, KT),
            "wk": sbl(Wk[:, sl], KT),
            "wv": sbl(Wv[:, sl], KT),
            "wo": sbl(Wo[sl, :], MT),
            "bq": np.ascontiguousarray((bq[sl] * SCALE).reshape(MT, P).T),
            "msk": masks.astype(bf16),
            "idn": np.eye(P, dtype=np.float32).astype(bf16),
            "neg": ((1.0 - masks) * -100000.0).astype(bf16),
        })
    return in_maps


def run(inputs, trace=False):
    from concourse.bass_utils import run_bass_kernel_spmd

    nc = _get_program()
    in_maps = make_in_maps(**inputs)
    res = run_bass_kernel_spmd(nc, in_maps, list(range(8)), trace=trace)
    # softmax rows sum to 1, so the V bias adds bv to every attention
    # output exactly; fold bv @ Wo into the host-side output bias
    bo_eff = inputs["bo"] + inputs["bv"].astype(np.float64) @ \
        inputs["Wo"].astype(np.float64)
    bo_eff = bo_eff.astype(np.float32)
    out = np.empty((B, T, D), dtype=np.float32)
    for b in range(B):
        out[b] = res.results[2 * b]["out"] + res.results[2 * b + 1]["out"] \
            + bo_eff
    return out, res


def kernel(**inputs):
    inputs = {k: np.asarray(v) for k, v in inputs.items()}
    out, _ = run(inputs)
    return out

